# revision 1
# baseline (speedup 1.0000x reference)
"""Trainium2 Bass kernel for nn_Discriminator_77687368450470.

8-core SPMD strategy:
  - l0 (4096x4096 linear): output-feature-sharded (512 cols/core, all 256
    batches), then AllToAll #1 -> batch-sharded h0 (4096 len x 32 batches).
  - conv1/conv2: batch-parallel (32 batches/core), conv as matmul with strided
    access patterns (no materialized im2col for conv2).
  - AllToAll #2: re-shard conv2 output to channel-sharded (32 ch x all 256
    batches per core) => BatchNorm stats fully local (no BN collective) and
    each core owns an 8192-wide slice of the l1 contraction (reads only 1/8
    of the 256MB l1 weight).
  - l1: contraction-sharded partial matmul, AllReduce (1MB) of partials.
  - Every core then holds full feat (256,1024); computes M = feat @ T
    redundantly (cheap) and its 32-row slice of the pairwise exp-L1 block.
  - Output: sigmoid(z @ out_w.T + out_b) for the local 32 rows; host concat.

All weights are pre-sliced/pre-transposed/bf16-cast on the host so every
device DMA is contiguous or near-contiguous.
"""

import numpy as np
import ml_dtypes

# ---------------- constants (hardcoded problem shapes) ----------------
NCORES = 8
B = 256            # global batch
BL = B // NCORES   # local batch = 32
NS = 4096          # signal len
NF = 4096          # l0 out features
MSL = NF // NCORES # l0 cols per core = 512
L1 = 1024          # conv1 out len
L2 = 256           # conv2 out len
C1 = 128           # conv1 out channels
C2 = 256           # conv2 out channels
CHL = C2 // NCORES # conv2 channels per core = 32
IN_F = 1024        # l1 out features
KSL = C2 * L2 // NCORES  # l1 contraction slice = 8192
OUT_F = 128        # batch-disc features
KD = 16            # batch-disc kernel dims
PAD = 6
NP_PIECE = 128     # conv1 positions per im2col piece
NA = 260           # h1pad p_pad slots / 4  (p_pad in [0, 1040))
F32 = np.float32
BF16 = ml_dtypes.bfloat16

_CACHE = {}


def _build_program(upto=99):
    import concourse.bass as bass
    import concourse.mybir as mybir
    import concourse.tile as tile
    from concourse import bacc
    from concourse.bass import ds
    from contextlib import ExitStack

    dt = mybir.dt
    AF = mybir.ActivationFunctionType
    ALU = mybir.AluOpType

    nc = bacc.Bacc(num_devices=NCORES)

    # ---------------- I/O declarations ----------------
    p_xT = nc.declare_dram_parameter("xT", [NS, B], dt.bfloat16, isOutput=False)
    p_l0wT = nc.declare_dram_parameter("l0wT", [NS, MSL], dt.bfloat16, isOutput=False)
    p_l0b = nc.declare_dram_parameter("l0b", [MSL], dt.float32, isOutput=False)
    p_c1wT = nc.declare_dram_parameter("c1wT", [KD, C1], dt.bfloat16, isOutput=False)
    p_w2T = nc.declare_dram_parameter("w2T", [KD, C1, C2], dt.bfloat16, isOutput=False)
    p_bng = nc.declare_dram_parameter("bng", [CHL], dt.float32, isOutput=False)
    p_bnb = nc.declare_dram_parameter("bnb", [CHL], dt.float32, isOutput=False)
    p_l1wT = nc.declare_dram_parameter("l1wT", [KSL, IN_F], dt.bfloat16, isOutput=False)
    p_l1b = nc.declare_dram_parameter("l1b", [IN_F], dt.float32, isOutput=False)
    p_Tp = nc.declare_dram_parameter("Tp", [IN_F, KD * OUT_F], dt.bfloat16,
                                     isOutput=False)
    p_wf = nc.declare_dram_parameter("wf", [IN_F], dt.bfloat16, isOutput=False)
    p_wo = nc.declare_dram_parameter("wo", [OUT_F], dt.bfloat16, isOutput=False)
    p_outb = nc.declare_dram_parameter("outb", [1], dt.float32, isOutput=False)
    p_out = nc.declare_dram_parameter("out", [BL, 1], dt.float32, isOutput=True)

    RG = [list(range(NCORES))]

    done = [False]

    with tile.TileContext(nc) as tc, ExitStack() as ctx:

        def dummy_out(pool, src):
            # write p_out from live data so truncated variants keep all work
            r = pool.tile([1, BL], dt.float32, name="dummyres")
            nc.scalar.activation(r[:, :], src, AF.Copy, bias=0.0, scale=1.0)
            nc.sync.dma_start(out=p_out[:, :], in_=r[0:1, :])
            done[0] = True
        dram = ctx.enter_context(tc.tile_pool(name="dram", bufs=1, space="DRAM"))
        a2a1_in = dram.tile([NCORES, MSL, BL], dt.bfloat16)
        h0pad = dram.tile([NS + 2 * PAD, BL], dt.bfloat16)
        a2a2_in = dram.tile([NCORES, BL, CHL, L2], dt.bfloat16)
        a2a2_out = dram.tile([NCORES, BL, CHL, L2], dt.bfloat16)
        ar_in = dram.tile([B, IN_F], dt.float32)
        ar_out = dram.tile([B, IN_F], dt.float32)

        const_pool = ctx.enter_context(tc.tile_pool(name="const", bufs=1))

        class _Stop(Exception):
            pass

        try:
            # =========== Phase 1: l0 = x @ l0_w.T (my 512-col slice, all B) =====
            NKT0 = NS // 128   # 32
            NMT0 = MSL // 128  # 4
            h0T = []
            with tc.tile_pool(name="ph1", bufs=4) as ph1, \
                 tc.tile_pool(name="ph1o", bufs=1) as ph1o, \
                 tc.tile_pool(name="psum0", bufs=NMT0, space="PSUM") as psum0:
                l0b_sb = const_pool.tile([128, NMT0], dt.float32)
                nc.sync.dma_start(out=l0b_sb[:, :],
                                  in_=p_l0b.ap().rearrange("(a p) -> p a", p=128))
                ps0 = [psum0.tile([128, B], dt.float32, tag="ps0", name=f"ps0_{i}")
                       for i in range(NMT0)]
                for kt in range(NKT0):
                    xt = ph1.tile([128, B], dt.bfloat16, tag="xt", name=f"xt{kt}")
                    nc.sync.dma_start(out=xt[:, :], in_=p_xT[128 * kt:128 * (kt + 1), :])
                    wt = ph1.tile([128, MSL], dt.bfloat16, tag="l0w", name=f"l0w{kt}")
                    nc.sync.dma_start(out=wt[:, :], in_=p_l0wT[128 * kt:128 * (kt + 1), :])
                    for mt in range(NMT0):
                        nc.tensor.matmul(
                            ps0[mt][:, :],
                            lhsT=wt[:, 128 * mt:128 * (mt + 1)],
                            rhs=xt[:, :],
                            start=(kt == 0),
                            stop=(kt == NKT0 - 1),
                        )
                for mt in range(NMT0):
                    o = ph1o.tile([128, B], dt.bfloat16, tag=f"h0T{mt}", name=f"h0T_{mt}")
                    nc.scalar.activation(o[:, :], ps0[mt][:, :], AF.Identity,
                                         bias=l0b_sb[:, mt:mt + 1], scale=1.0)
                    h0T.append(o)

                # ------- AllToAll #1 (len-sharded -> batch-sharded) -------
                for mt in range(NMT0):
                    for j in range(NCORES):
                        nc.sync.dma_start(
                            out=a2a1_in[j, 128 * mt:128 * (mt + 1), :],
                            in_=h0T[mt][:, BL * j:BL * (j + 1)],
                        )
                zpad = const_pool.tile([PAD, BL], dt.bfloat16)
                nc.vector.memset(zpad[:, :], 0.0)
                nc.sync.dma_start(out=h0pad[0:PAD, :], in_=zpad[:, :])
                nc.sync.dma_start(out=h0pad[PAD + NS:PAD + NS + PAD, :], in_=zpad[:, :])
                nc.gpsimd.collective_compute(
                    "AllToAll", ALU.bypass, replica_groups=RG,
                    ins=[a2a1_in[:, :, :].opt()],
                    outs=[h0pad[PAD:PAD + NS, :].opt()],
                )

            if upto == 1:
                with tc.tile_pool(name="dp1", bufs=1) as dp1:
                    t1_ = dp1.tile([1, BL], dt.bfloat16)
                    nc.sync.dma_start(out=t1_[:, :], in_=h0pad[0:1, :])
                    dummy_out(dp1, t1_[0:1, :])
                    raise _Stop()

            # =========== Phase 3: conv1 (1->128ch, k16 s4 p6) + lrelu ===========
            # h1pad layout [128 ic, NA a, 4 r, BL b]; p_pad = 4a + r; p = p_pad - 6
            h1_pool = ctx.enter_context(tc.tile_pool(name="h1", bufs=1))
            h1pad = h1_pool.tile([128, NA, 4, BL], dt.bfloat16)
            h1flat = h1pad.rearrange("p a r b -> p (a r b)")

            def im2col_src(piece):
                # rhs1[k, p, b] = h0pad[4p + k + 4*NP_PIECE*piece, b]
                base = 4 * NP_PIECE * piece
                return bass.AP(tensor=h0pad.tensor,
                               offset=h0pad.offset + base * BL,
                               ap=[[BL, KD], [4 * BL, NP_PIECE], [1, BL]])

            with tc.tile_pool(name="ph3", bufs=2) as ph3, \
                 tc.tile_pool(name="psum1", bufs=4, space="PSUM") as psum1:
                c1w_sb = const_pool.tile([KD, C1], dt.bfloat16)
                nc.sync.dma_start(out=c1w_sb[:, :], in_=p_c1wT[:, :])
                nc.vector.memset(h1flat[:, 0:PAD * BL], 0.0)
                nc.vector.memset(h1flat[:, (PAD + L1) * BL:NA * 4 * BL], 0.0)
                for piece in range(L1 // NP_PIECE):  # 8
                    rhs1 = ph3.tile([KD, NP_PIECE, BL], dt.bfloat16, tag="rhs1",
                                    name=f"rhs1_{piece}")
                    nc.sync.dma_start(out=rhs1[:, :, :], in_=im2col_src(piece))
                    for s in range(NP_PIECE // 16):  # 8 chunks of N=512
                        ps1 = psum1.tile([128, 512], dt.float32, tag="ps1",
                                         name=f"ps1_{piece}_{s}")
                        nc.tensor.matmul(
                            ps1[:, :],
                            lhsT=c1w_sb[:, :],
                            rhs=rhs1[:, 16 * s:16 * (s + 1), :],
                            start=True, stop=True,
                        )
                        off = (PAD + NP_PIECE * piece + 16 * s) * BL
                        c1t = ph3.tile([128, 512], dt.bfloat16, tag="c1t",
                                       name=f"c1t_{piece}_{s}")
                        nc.scalar.activation(c1t[:, :], ps1[:, :], AF.Copy,
                                             bias=0.0, scale=1.0)
                        nc.vector.scalar_tensor_tensor(
                            out=h1flat[:, off:off + 512],
                            in0=c1t[:, :], scalar=0.2, in1=c1t[:, :],
                            op0=ALU.mult, op1=ALU.max,
                        )

            if upto == 2:
                with tc.tile_pool(name="dp2", bufs=1) as dp2:
                    dummy_out(dp2, h1flat[0:1, 0:BL])
                    raise _Stop()

            # =========== Phase 4: conv2 (128->256ch, k16 s4 p6) + A2A#2 =========
            with tc.tile_pool(name="ph4w", bufs=1) as ph4w, \
                 tc.tile_pool(name="ph4o", bufs=1) as ph4o, \
                 tc.tile_pool(name="psum2", bufs=4, space="PSUM") as psum2:
                w2_sb = ph4w.tile([128, KD, C2], dt.bfloat16)
                nc.sync.dma_start(out=w2_sb[:, :, :],
                                  in_=p_w2T[:, :, :].rearrange("k i o -> i k o"))
                c2sb = [ph4o.tile([128, L2, BL], dt.bfloat16, tag=f"c2sb{h}",
                                  name=f"c2sb_{h}") for h in range(2)]
                for half in range(2):
                    for pc in range(L2 // 16):  # 16 chunks of 16 p2
                        ps2 = psum2.tile([128, 512], dt.float32, tag="ps2",
                                         name=f"ps2_{half}_{pc}")
                        for k in range(KD):
                            a0 = 16 * pc + k // 4
                            r0 = k % 4
                            nc.tensor.matmul(
                                ps2[:, :],
                                lhsT=w2_sb[:, k, 128 * half:128 * (half + 1)],
                                rhs=h1pad[:, a0:a0 + 16, r0, :],
                                start=(k == 0), stop=(k == KD - 1),
                            )
                        nc.scalar.activation(
                            c2sb[half][:, 16 * pc:16 * (pc + 1), :],
                            ps2[:, :], AF.Copy, bias=0.0, scale=1.0,
                        )
                for j in range(NCORES):
                    half = j // 4
                    part0 = CHL * (j % 4)
                    nc.sync.dma_start(
                        out=a2a2_in[j, :, :, :].rearrange("b c l -> c l b"),
                        in_=c2sb[half][part0:part0 + CHL, :, :],
                    )
                nc.gpsimd.collective_compute(
                    "AllToAll", ALU.bypass, replica_groups=RG,
                    ins=[a2a2_in[:, :, :, :].opt()],
                    outs=[a2a2_out[:, :, :, :].opt()],
                )

            if upto == 3:
                with tc.tile_pool(name="dp3", bufs=1) as dp3:
                    t3_ = dp3.tile([1, BL], dt.bfloat16)
                    nc.sync.dma_start(out=t3_[:, :], in_=a2a2_out[0, 0:1, 0, 0:BL])
                    dummy_out(dp3, t3_[0:1, :])
                    raise _Stop()

            # =========== Phase 5: assemble l1-lhsT + BatchNorm + lrelu ==========
            # lhsT layout [128 p2l, CHL c, 2 h, B b]; K = c*256 + h*128 + p2l
            ph5 = ctx.enter_context(tc.tile_pool(name="ph5", bufs=1))
            lhsT_raw = ph5.tile([128, B, CHL, 2], dt.bfloat16)
            lhsT_bn = ph5.tile([128, B, CHL, 2], dt.bfloat16)
            with tc.tile_pool(name="ph5t", bufs=1) as ph5t, \
                 tc.tile_pool(name="psum5", bufs=1, space="PSUM") as psum5:
                for j in range(NCORES):
                    for h in range(2):
                        nc.sync.dma_start(
                            out=lhsT_raw[:, BL * j:BL * (j + 1), :, h].opt(),
                            in_=a2a2_out[j, :, :, 128 * h:128 * (h + 1)]
                            .rearrange("b c q -> q b c").opt(),
                        )
                stats = ph5t.tile([128, 2 * CHL], dt.float32)
                scr = ph5t.tile([128, 2 * B], dt.bfloat16)
                for c in range(CHL):
                    nc.vector.tensor_reduce(
                        out=stats[:, c:c + 1], in_=lhsT_raw[:, :, c, :],
                        axis=mybir.AxisListType.XY, op=ALU.add,
                    )
                    nc.scalar.activation(
                        scr[:, :].rearrange("p (b h) -> p b h", h=2),
                        lhsT_raw[:, :, c, :],
                        AF.Square, accum_out=stats[:, CHL + c:CHL + c + 1],
                    )
                ones = const_pool.tile([128, 1], dt.float32)
                nc.vector.memset(ones[:, :], 1.0)
                ps_st = psum5.tile([128, 2 * CHL], dt.float32)
                nc.tensor.matmul(ps_st[0:1, :], lhsT=ones[:, :], rhs=stats[:, :],
                                 start=True, stop=True)
                CNT = 1.0 / float(128 * 2 * B)
                mean = ph5t.tile([1, CHL], dt.float32)
                ex2 = ph5t.tile([1, CHL], dt.float32)
                var = ph5t.tile([1, CHL], dt.float32)
                sd = ph5t.tile([1, CHL], dt.float32)
                inv = ph5t.tile([1, CHL], dt.float32)
                bng_sb = ph5t.tile([1, CHL], dt.float32)
                bnb_sb = ph5t.tile([1, CHL], dt.float32)
                nc.sync.dma_start(out=bng_sb[:, :], in_=p_bng.ap().unsqueeze(0))
                nc.sync.dma_start(out=bnb_sb[:, :], in_=p_bnb.ap().unsqueeze(0))
                nc.vector.tensor_scalar_mul(mean[:, :], ps_st[0:1, 0:CHL], CNT)
                nc.vector.tensor_scalar_mul(ex2[:, :], ps_st[0:1, CHL:2 * CHL], CNT)
                nc.vector.tensor_tensor(var[:, :], mean[:, :], mean[:, :], op=ALU.mult)
                nc.vector.tensor_tensor(var[:, :], ex2[:, :], var[:, :], op=ALU.subtract)
                nc.vector.tensor_scalar_add(sd[:, :], var[:, :], 1e-5)
                nc.scalar.activation(var[:, :], sd[:, :], AF.Sqrt, bias=0.0, scale=1.0)
                nc.vector.reciprocal(inv[:, :], var[:, :])
                ab = ph5t.tile([1, 2 * CHL], dt.float32)
                nc.vector.tensor_tensor(ab[:, 0:CHL], bng_sb[:, :], inv[:, :], op=ALU.mult)
                nc.vector.tensor_tensor(ex2[:, :], mean[:, :], ab[:, 0:CHL], op=ALU.mult)
                nc.vector.tensor_tensor(ab[:, CHL:2 * CHL], bnb_sb[:, :], ex2[:, :],
                                        op=ALU.subtract)
                abb = ph5t.tile([128, 2 * CHL], dt.float32)
                ones_r = ph5t.tile([1, 128], dt.float32)
                nc.vector.memset(ones_r[:, :], 1.0)
                ps_bc = psum5.tile([128, 2 * CHL], dt.float32, tag="ps_bc")
                nc.tensor.matmul(ps_bc[:, :], lhsT=ones_r[0:1, :], rhs=ab[0:1, :],
                                 start=True, stop=True)
                nc.vector.tensor_copy(abb[:, :], ps_bc[:, :])
                scr2 = ph5t.tile([128, 2 * B], dt.float32)
                for c in range(CHL):
                    scr2v = scr2[:, :].rearrange("p (b h) -> p b h", h=2)
                    nc.scalar.activation(
                        scr2v, lhsT_raw[:, :, c, :],
                        AF.Identity, bias=abb[:, CHL + c:CHL + c + 1],
                        scale=abb[:, c:c + 1],
                    )
                    nc.vector.scalar_tensor_tensor(
                        out=lhsT_bn[:, :, c, :],
                        in0=scr2v, scalar=0.2, in1=scr2v,
                        op0=ALU.mult, op1=ALU.max,
                    )

            if upto == 4:
                with tc.tile_pool(name="dp4", bufs=1) as dp4:
                    dummy_out(dp4, lhsT_bn[0:1, 0:BL, 0, 0])
                    raise _Stop()

            # =========== Phase 6: l1 partial matmul + AllReduce + lrelu =========
            NT1 = KSL // 128  # 64
            NMT = IN_F // 128  # 8
            featT = []
            ftpool = ctx.enter_context(tc.tile_pool(name="featT", bufs=1))
            with tc.tile_pool(name="ph6", bufs=6) as ph6, \
                 tc.tile_pool(name="ph6s", bufs=1) as ph6s, \
                 tc.tile_pool(name="psum6", bufs=4, space="PSUM") as psum6:
                ps6 = [psum6.tile([128, 512], dt.float32, tag="ps6", name=f"ps6_{i}")
                       for i in range(4)]
                for t in range(NT1):
                    wl = ph6.tile([128, IN_F], dt.bfloat16, tag="l1w", name=f"l1w{t}")
                    nc.sync.dma_start(out=wl[:, :], in_=p_l1wT[128 * t:128 * (t + 1), :])
                    c, h = t // 2, t % 2
                    for bt in range(2):
                        for mc in range(2):
                            nc.tensor.matmul(
                                ps6[2 * bt + mc][:, :],
                                lhsT=lhsT_bn[:, 128 * bt:128 * (bt + 1), c, h],
                                rhs=wl[:, 512 * mc:512 * (mc + 1)],
                                start=(t == 0), stop=(t == NT1 - 1),
                            )
                for bt in range(2):
                    fp = ph6s.tile([128, IN_F], dt.float32, tag=f"fp{bt}",
                                   name=f"fp_{bt}")
                    for mc in range(2):
                        nc.scalar.activation(fp[:, 512 * mc:512 * (mc + 1)],
                                             ps6[2 * bt + mc][:, :], AF.Copy,
                                             bias=0.0, scale=1.0)
                    nc.sync.dma_start(out=ar_in[128 * bt:128 * (bt + 1), :],
                                      in_=fp[:, :])
                nc.gpsimd.collective_compute(
                    "AllReduce", ALU.add, replica_groups=RG,
                    ins=[ar_in[:, :].opt()], outs=[ar_out[:, :].opt()],
                )
                l1b_sb = const_pool.tile([128, NMT], dt.float32)
                nc.sync.dma_start(out=l1b_sb[:, :],
                                  in_=p_l1b.ap().rearrange("(a p) -> p a", p=128))
                scr3 = ph6s.tile([128, B], dt.float32)
                for mt in range(NMT):
                    raw = ph6.tile([128, B], dt.float32, tag="ftraw", name=f"ftr{mt}")
                    nc.sync.dma_start(
                        out=raw[:, :],
                        in_=ar_out[:, :].rearrange("b (a p) -> p a b", p=128)[:, mt, :],
                    )
                    nc.scalar.activation(scr3[:, :], raw[:, :], AF.Identity,
                                         bias=l1b_sb[:, mt:mt + 1], scale=1.0)
                    ft = ftpool.tile([128, B], dt.bfloat16, tag=f"ft{mt}",
                                     name=f"ft_{mt}")
                    nc.vector.scalar_tensor_tensor(
                        out=ft[:, :], in0=scr3[:, :], scalar=0.2, in1=scr3[:, :],
                        op0=ALU.mult, op1=ALU.max,
                    )
                    featT.append(ft)

            if upto == 5:
                with tc.tile_pool(name="dp5", bufs=1) as dp5:
                    dummy_out(dp5, featT[0][0:1, 0:BL])
                    raise _Stop()

            # =========== Phase 7: M = feat @ Tp -> [128 oc, B j, KD kd] =========
            ph7 = ctx.enter_context(tc.tile_pool(name="ph7", bufs=1))
            M_sb = ph7.tile([128, B, KD], dt.bfloat16)
            with tc.tile_pool(name="ph7w", bufs=1) as ph7w, \
                 tc.tile_pool(name="psum7", bufs=4, space="PSUM") as psum7:
                tps = []
                for mt in range(NMT):
                    tp = ph7w.tile([128, KD * OUT_F], dt.bfloat16, tag=f"tp{mt}",
                                   name=f"tp_{mt}")
                    nc.sync.dma_start(out=tp[:, :], in_=p_Tp[128 * mt:128 * (mt + 1), :])
                    tps.append(tp)
                for c in range(KD):  # col chunk c = kd index (host permuted T)
                    ps7 = psum7.tile([128, B], dt.float32, tag="ps7", name=f"ps7_{c}")
                    for mt in range(NMT):
                        nc.tensor.matmul(
                            ps7[:, :],
                            lhsT=tps[mt][:, 128 * c:128 * (c + 1)],
                            rhs=featT[mt][:, :],
                            start=(mt == 0), stop=(mt == NMT - 1),
                        )
                    nc.scalar.activation(M_sb[:, :, c], ps7[:, :], AF.Copy,
                                         bias=0.0, scale=1.0)

            if upto == 6:
                with tc.tile_pool(name="dp6", bufs=1) as dp6:
                    dummy_out(dp6, M_sb[0:1, 0:BL, 0])
                    raise _Stop()

            # =========== Phase 8: pairwise exp-L1 block (my 32 rows) ============
            ob_pool = ctx.enter_context(tc.tile_pool(name="ob", bufs=1))
            o_b = ob_pool.tile([128, BL], dt.float32)
            pid_v = nc.vector.partition_id()
            with tc.tile_pool(name="ph8", bufs=3) as ph8, \
                 tc.tile_pool(name="ph8m", bufs=1) as ph8m:
                mloc = ph8m.tile([128, BL, KD], dt.bfloat16)
                nc.vector.tensor_copy(
                    mloc.rearrange("p b k -> p (b k)"),
                    M_sb.rearrange("p b k -> p (b k)")[:, ds(pid_v * (BL * KD), BL * KD)],
                )
                for i in range(BL):
                    dtl = ph8.tile([128, B, KD], dt.bfloat16, tag="dt", name=f"dt{i}")
                    a_b, b_b = bass.broadcast_tensor_aps(M_sb[:, :, :], mloc[:, i:i + 1, :])
                    nc.vector.tensor_tensor(dtl[:, :, :], a_b, b_b, op=ALU.subtract)
                    nrm = ph8.tile([128, B], dt.float32, tag="nrm", name=f"nrm{i}")
                    nc.vector.tensor_reduce(
                        out=nrm[:, :], in_=dtl[:, :, :], axis=mybir.AxisListType.X,
                        op=ALU.add, apply_absolute_value=True,
                    )
                    esc = ph8.tile([128, B], dt.bfloat16, tag="esc", name=f"esc{i}")
                    nc.scalar.activation(esc[:, :], nrm[:, :], AF.Exp, scale=-1.0,
                                         accum_out=o_b[:, i:i + 1])

            if upto == 7:
                with tc.tile_pool(name="dp7", bufs=1) as dp7:
                    dummy_out(dp7, o_b[0:1, :])
                    raise _Stop()

            # =========== Phase 9: output head ===================================
            with tc.tile_pool(name="ph9", bufs=1) as ph9, \
                 tc.tile_pool(name="psum9", bufs=2, space="PSUM") as psum9:
                obb = ph9.tile([128, BL], dt.bfloat16)
                nc.vector.tensor_scalar_add(obb[:, :], o_b[:, :], -1.0)
                wf_sb = ph9.tile([128, NMT], dt.bfloat16)
                nc.sync.dma_start(out=wf_sb[:, :],
                                  in_=p_wf.ap().rearrange("(a p) -> p a", p=128))
                wo_sb = ph9.tile([128, 1], dt.bfloat16)
                nc.sync.dma_start(out=wo_sb[:, :], in_=p_wo.ap().unsqueeze(1))
                outb_sb = ph9.tile([1, 1], dt.float32)
                nc.sync.dma_start(out=outb_sb[:, :],
                                  in_=p_outb.ap().unsqueeze(0))
                ps_f = psum9.tile([128, B], dt.float32)
                for mt in range(NMT):
                    nc.tensor.matmul(ps_f[0:1, :], lhsT=wf_sb[:, mt:mt + 1],
                                     rhs=featT[mt][:, :],
                                     start=(mt == 0), stop=(mt == NMT - 1))
                ps_o = psum9.tile([128, BL], dt.float32)
                nc.tensor.matmul(ps_o[0:1, :], lhsT=wo_sb[:, :], rhs=obb[:, :],
                                 start=True, stop=True)
                s_ob = ph9.tile([1, BL], dt.float32)
                nc.scalar.activation(s_ob[:, :], ps_o[0:1, :], AF.Copy,
                                     bias=0.0, scale=1.0)
                logit = ph9.tile([1, BL], dt.float32)
                pid_v2 = nc.vector.partition_id()
                nc.vector.tensor_tensor(
                    logit[:, :], ps_f[0:1, ds(pid_v2 * BL, BL)], s_ob[:, :],
                    op=ALU.add,
                )
                res = ph9.tile([1, BL], dt.float32)
                nc.scalar.activation(res[:, :], logit[:, :], AF.Sigmoid,
                                     bias=outb_sb[0:1, :], scale=1.0)
                nc.sync.dma_start(out=p_out[:, :], in_=res[0:1, :])

        except _Stop:
            pass
    nc.finalize()
    return nc


def _host_prep(inputs):
    x = np.asarray(inputs["x"], F32).reshape(B, NS)
    l0_w = np.asarray(inputs["l0_w"], F32)
    l0_b = np.asarray(inputs["l0_b"], F32)
    conv1_w = np.asarray(inputs["conv1_w"], F32)
    conv2_w = np.asarray(inputs["conv2_w"], F32)
    bn_g = np.asarray(inputs["bn_g"], F32)
    bn_b = np.asarray(inputs["bn_b"], F32)
    l1_w = np.asarray(inputs["l1_w"], F32)
    l1_b = np.asarray(inputs["l1_b"], F32)
    T = np.asarray(inputs["T"], F32)
    out_w = np.asarray(inputs["out_w"], F32)
    out_b = np.asarray(inputs["out_b"], F32)

    xT = x.T.astype(BF16, order='C')  # astype on the view: one-pass strided read + cast
    c1wT = conv1_w[:, 0, :].T.astype(BF16, order='C')
    w2T = conv2_w.transpose(2, 1, 0).astype(BF16, order='C')
    l1b = np.ascontiguousarray(l1_b).astype(F32)
    Tp = T.transpose(0, 2, 1).astype(BF16, order='C').reshape(IN_F, KD * OUT_F)
    wf = out_w[0, :IN_F].astype(BF16)
    wo = out_w[0, IN_F:].astype(BF16)
    outb = np.ascontiguousarray(out_b).astype(F32)

    in_maps = []
    for k in range(NCORES):
        msl = slice(MSL * k, MSL * (k + 1))
        chsl = slice(CHL * k, CHL * (k + 1))
        ksl = slice(KSL * k, KSL * (k + 1))
        in_maps.append({
            "xT": xT,
            "l0wT": l0_w[msl, :].T.astype(BF16, order='C'),
            "l0b": np.ascontiguousarray(l0_b[msl]).astype(F32),
            "c1wT": c1wT,
            "w2T": w2T,
            "bng": np.ascontiguousarray(bn_g[chsl]).astype(F32),
            "bnb": np.ascontiguousarray(bn_b[chsl]).astype(F32),
            "l1wT": l1_w[:, ksl].T.astype(BF16, order='C'),
            "l1b": l1b,
            "Tp": Tp,
            "wf": wf,
            "wo": wo,
            "outb": outb,
        })
    return in_maps


def kernel(**inputs) -> np.ndarray:
    from concourse.bass_utils import run_bass_kernel_spmd

    if "nc" not in _CACHE:
        _CACHE["nc"] = _build_program()
    nc = _CACHE["nc"]
    in_maps = _host_prep(inputs)
    res = run_bass_kernel_spmd(nc, in_maps, core_ids=list(range(NCORES)))
    outs = [np.asarray(res.results[k]["out"], F32) for k in range(NCORES)]
    return np.concatenate(outs, axis=0).reshape(B, 1)

